# revision 20
# baseline (speedup 1.0000x reference)
"""Trainium2 Bass kernel for nn_ArnDiffRnnAgent (dense_mlp, 8-core data parallel).

Strategy (v5)
-------------
Pure data parallel: batch 8192 split into 8 shards of 1024 rows, one per
NeuronCore; small weights replicated.  Activations live in SBUF
feature-major ([feature, batch]) so every GEMM maps directly onto the
TensorEngine.  Compute dtype bf16 with fp32 PSUM accumulation.

ALL layout conversion runs on the TensorEngine (PE transposes):
 - inputs:  f32 natural loads -> f32r PE-transpose (1.5 cyc/row) -> PSUM ->
   bf16 SBUF eviction on ScalarE/VectorE
 - outputs: bf16 PE-transpose (1 cyc/row) -> PSUM -> f32 SBUF eviction ->
   plain contiguous f32 DMA stores
The DMA engines only ever do clean contiguous loads/stores (the DMA-XBAR
transpose path measured ~150 GB/s effective, starved the PE, and corrupts
(col%16>=8, even-row) cells for these shapes).

Host-side weight prep (parameter layout prep, amortized in any real use):
 - weights pre-transposed/tiled to [128, k_tiles, M] bf16
 - enemy e2-layer folded into GRU input weights:
      gi = e2 @ wih.T + bih,  e2 = relu1 @ W_e2.T + b_e2
   => gi = relu1 @ (wih @ W_e2).T + (wih @ b_e2 + bih)
 - r/z gates of each GRU computed as one K=512 GEMM over [x_side | h_side]
"""
import sys
sys.path.insert(0, "/opt/trn_rl_repo")

import numpy as np
import ml_dtypes

import concourse.bass as bass  # noqa: F401
import concourse.tile as tile
from concourse import bacc, mybir
from concourse.bass_utils import run_bass_kernel_spmd

BF16 = mybir.dt.bfloat16
F32 = mybir.dt.float32
F32R = mybir.dt.float32r
AF = mybir.ActivationFunctionType
ALU = mybir.AluOpType
bf16 = ml_dtypes.bfloat16

N_CORES = 8
B = 8192
IN = 1504
H = 256
E = 64
MOVE = 16
N_EN = 16
HID = (1 + N_EN) * H  # 4352

_CACHE = {}


# --------------------------------------------------------------------------
# host-side weight prep
# --------------------------------------------------------------------------
def _ktile(wt):
    """[K, M] (f32) -> [128, K//128, M] bf16 host array (K padded to 128s)."""
    K = wt.shape[0]
    Kp = ((K + 127) // 128) * 128
    if Kp != K:
        wt = np.concatenate([wt, np.zeros((Kp - K, wt.shape[1]), np.float32)], 0)
    return np.ascontiguousarray(
        wt.reshape(Kp // 128, 128, wt.shape[1]).transpose(1, 0, 2)
    ).astype(bf16)


def _btile(b):
    """[M] f32 -> [128, M//128] f32."""
    return np.ascontiguousarray(b.reshape(-1, 128).T).astype(np.float32)


def prep_weights(i):
    w = {}
    f32 = np.float32
    # env layer1: k-tile 11 matches the overlapped transpose source
    # (x cols 1376:1504): rows 0:32 of that tile are zero (cols 1376:1408
    # are already covered by k-tile 10).
    w1 = i["W_env1"].astype(f32).T                           # [1504, 256]
    w1p = np.zeros((1536, 256), f32)
    w1p[0:1408] = w1[0:1408]
    w1p[1440:1536] = w1[1408:1504]
    w["wenv1"] = np.ascontiguousarray(
        w1p.reshape(12, 128, 256).transpose(1, 0, 2)).astype(bf16)
    w["benv1"] = _btile(i["b_env1"].astype(f32))
    w["wenv2"] = _ktile(i["W_env2"].astype(f32).T)           # [128,2,256]
    w["benv2"] = _btile(i["b_env2"].astype(f32))

    def gru(tag, wih, whh, bih, bhh):
        w["wrz" + tag] = _ktile(
            np.concatenate([wih[:512].T, whh[:512].T], 0))   # [128,4,512]
        w["brz" + tag] = _btile(bih[:512] + bhh[:512])
        w["wgin" + tag] = _ktile(wih[512:].T)                # [128,2,256]
        w["bgin" + tag] = _btile(bih[512:])
        w["wghn" + tag] = _ktile(whh[512:].T)
        w["bghn" + tag] = _btile(bhh[512:])

    gru("e", i["wih_env"].astype(f32), i["whh_env"].astype(f32),
        i["bih_env"].astype(f32), i["bhh_env"].astype(f32))

    for tag in ("a", "b"):
        wih = i["wih_e" + tag].astype(f32)
        whh = i["whh_e" + tag].astype(f32)
        bih = i["bih_e" + tag].astype(f32)
        bhh = i["bhh_e" + tag].astype(f32)
        We2 = i["W_e2" + tag].astype(f32)
        be2 = i["b_e2" + tag].astype(f32)
        Wgi = wih @ We2                                       # [768,256]
        bgi = wih @ be2 + bih
        gru(tag, Wgi, whh, bgi, bhh)
        # e1 weights duplicated in both partition halves (odd enemies sit at
        # base_partition 64 and matmul requires equal operand bases)
        e1t = i["W_e1" + tag].astype(f32).T                   # [64,256]
        w["we1" + tag] = np.ascontiguousarray(
            np.concatenate([e1t, e1t], 0)).astype(bf16)       # [128,256]
        w["be1" + tag] = _btile(i["b_e1" + tag].astype(f32))

    w["wwo"] = _ktile(i["W_wo"].astype(f32).T)                # [128,2,6]
    w["bwo"] = np.ascontiguousarray(i["b_wo"].astype(f32).reshape(6, 1))
    sel = np.zeros((128, 64), f32)
    for j in range(8):
        sel[:, j * 8 + j] = 1.0
    w["sel"] = sel.astype(bf16)
    w["ident"] = np.eye(128, dtype=f32)
    w["ident16"] = np.eye(128, dtype=f32).astype(bf16)
    return w


WEIGHT_SPECS = {
    "wenv1": ([128, 12, 256], BF16), "benv1": ([128, 2], F32),
    "wenv2": ([128, 2, 256], BF16), "benv2": ([128, 2], F32),
    "wwo": ([128, 2, 6], BF16), "bwo": ([6, 1], F32),
    "sel": ([128, 64], BF16), "ident": ([128, 128], F32R),
    "ident16": ([128, 128], BF16),
}
for _t in ("e", "a", "b"):
    WEIGHT_SPECS["wrz" + _t] = ([128, 4, 512], BF16)
    WEIGHT_SPECS["brz" + _t] = ([128, 4], F32)
    WEIGHT_SPECS["wgin" + _t] = ([128, 2, 256], BF16)
    WEIGHT_SPECS["bgin" + _t] = ([128, 2], F32)
    WEIGHT_SPECS["wghn" + _t] = ([128, 2, 256], BF16)
    WEIGHT_SPECS["bghn" + _t] = ([128, 2], F32)
for _t in ("a", "b"):
    WEIGHT_SPECS["we1" + _t] = ([128, 256], BF16)
    WEIGHT_SPECS["be1" + _t] = ([128, 2], F32)


# --------------------------------------------------------------------------
# device kernel builder
# --------------------------------------------------------------------------
def build(bc=1024, chunk=512):
    """bc = batch rows per core, chunk = moving free-dim per GEMM."""
    nc = bacc.Bacc("TRN2", target_bir_lowering=False, debug=False,
                   num_devices=N_CORES)

    ns = chunk // 128  # b-subtiles per chunk

    x = nc.dram_tensor("x", [bc, IN], F32, kind="ExternalInput")
    h = nc.dram_tensor("h", [bc, HID], F32, kind="ExternalInput")
    W = {k: nc.dram_tensor(k, s, d, kind="ExternalInput")
         for k, (s, d) in WEIGHT_SPECS.items()}

    q_out = nc.dram_tensor("q", [bc, 22], F32, kind="ExternalOutput")
    h_out = nc.dram_tensor("hidden", [bc, HID], F32, kind="ExternalOutput")

    nch = bc // chunk

    with tile.TileContext(nc) as tc:
        with (
            tc.tile_pool(name="wts", bufs=1) as wpool,
            tc.tile_pool(name="xn", bufs=1) as xn_pool,
            tc.tile_pool(name="hn", bufs=5) as hn_pool,
            tc.tile_pool(name="xt", bufs=2) as xt_pool,
            tc.tile_pool(name="ft", bufs=2) as ft_pool,
            tc.tile_pool(name="ht", bufs=5) as ht_pool,
            tc.tile_pool(name="ev", bufs=3) as ev_pool,
            tc.tile_pool(name="am", bufs=10) as am_pool,
            tc.tile_pool(name="outp", bufs=3) as out_pool,
            tc.tile_pool(name="psT", bufs=2, space="PSUM") as psT,
            tc.tile_pool(name="psA", bufs=2, space="PSUM") as psA,
            tc.tile_pool(name="psB", bufs=2, space="PSUM") as psB,
        ):
            # ---- weights into SBUF -------------------------------------
            wt = {}
            for k, (s, d) in WEIGHT_SPECS.items():
                wt[k] = wpool.tile(s, d, name="w_" + k)
                nc.sync.dma_start(wt[k][:], W[k][:])
            idt = wt["ident"]
            idt16 = wt["ident16"]

            def mm(psum_ap, lhs_ap, mov_ap, start, stop):
                nc.tensor.matmul(psum_ap, lhs_ap, mov_ap, start=start, stop=stop)

            def gemm(ps_tile, wkey, movs, m_tiles, n):
                wtile = wt[wkey]
                nk = len(movs)
                for m in range(m_tiles):
                    for ki, mov in enumerate(movs):
                        mm(ps_tile[:, m, :],
                           wtile[:, ki, 128 * m:128 * (m + 1)],
                           mov, ki == 0, ki == nk - 1)

            evict_ctr = [0]

            def evict(dst_ap, src_ap):
                evict_ctr[0] += 1
                if evict_ctr[0] % 2 == 0:
                    nc.scalar.activation(dst_ap, src_ap, AF.Identity)
                else:
                    nc.vector.tensor_copy(dst_ap, src_ap)

            # input transpose: ns x [128,128] f32r tiles -> bf16 [128, chunk]
            def pe_transpose_in(col_fn, dst_ap):
                pt = psT.tile([128, ns, 128], BF16, name="pt", tag="psT")
                for s in range(ns):
                    nc.tensor.transpose(pt[:, s, :], col_fn(s), idt16[:])
                evict(dst_ap, pt[:])

            # output transpose: src [128, 2, chunk] bf16 feature-major ->
            # f32 stores into out_dram[b0:b0+chunk, c0:c0+256]
            def pe_transpose_out(src, out_dram, b0, c0):
                for w0 in range(0, ns, 2):
                    sp = min(2, ns - w0)
                    po = psT.tile([128, 2, 2, 128], BF16, name="po", tag="psT")
                    for si in range(sp):
                        for k in range(2):
                            nc.tensor.transpose(
                                po[:, si, k, :],
                                src[:, k, 128 * (w0 + si):128 * (w0 + si + 1)],
                                idt16[:])
                    ho = out_pool.tile([128, 2, 256], F32, name="ho")
                    evict(ho[:, 0:sp, :], po[:, 0:sp, :, :])
                    for si in range(sp):
                        r0 = b0 + 128 * (w0 + si)
                        nc.sync.dma_start(
                            out=out_dram[r0:r0 + 128, c0:c0 + 256],
                            in_=ho[:, si, :])

            # ---- main loop ---------------------------------------------
            for c in range(nch):
                b0 = c * chunk

                # natural-layout f32 loads
                xn = xn_pool.tile([128, ns, IN], BF16, name="xn")
                for s in range(ns):
                    nc.gpsimd.dma_start(
                        out=xn[:, s, :],
                        in_=x[b0 + 128 * s:b0 + 128 * (s + 1), :])

                # transposed env input: [128, 12, chunk]
                xt = xt_pool.tile([128, 12, chunk], BF16, name="xt")
                for k in range(12):
                    c0 = 128 * k if k < 11 else IN - 128
                    pe_transpose_in(lambda s, c0=c0: xn[:, s, c0:c0 + 128],
                                    xt[:, k, :])

                # enemy features: 8 tiles covering 2 enemies each
                ft = ft_pool.tile([128, 8, chunk], BF16, name="ft")
                for j in range(8):
                    c0 = MOVE + 128 * j
                    pe_transpose_in(lambda s, c0=c0: xn[:, s, c0:c0 + 128],
                                    ft[:, j, :])

                def load_hT(col0, name):
                    hn = hn_pool.tile([128, ns, H], BF16, name="hn")
                    for s in range(ns):
                        nc.gpsimd.dma_start(
                            out=hn[:, s, :],
                            in_=h[b0 + 128 * s:b0 + 128 * (s + 1),
                                  col0:col0 + H])
                    hT = ht_pool.tile([128, 2, chunk], BF16, name=name)
                    for k in range(2):
                        pe_transpose_in(
                            lambda s, k=k: hn[:, s, 128 * k:128 * (k + 1)],
                            hT[:, k, :])
                    return hT

                # -------- shared GRU tail -------------------------------
                def gru_tail(tag, rz_ps, gin_ps, ghn_ps, hT, m_tiles=2):
                    rzs = ev_pool.tile([128, 2 * m_tiles, chunk], BF16, name="rzs")
                    for j in range(2 * m_tiles):
                        nc.scalar.activation(
                            rzs[:, j, :], rz_ps[j][:], AF.Sigmoid,
                            bias=wt["brz" + tag][:, j:j + 1])
                    t1 = ev_pool.tile([128, m_tiles, chunk], BF16, name="t1")
                    t2 = ev_pool.tile([128, m_tiles, chunk], BF16, name="t2")
                    nn = ev_pool.tile([128, m_tiles, chunk], BF16, name="nn")
                    hp = ev_pool.tile([128, m_tiles, chunk], BF16, name="hp")
                    for m in range(m_tiles):
                        nc.vector.scalar_tensor_tensor(
                            t1[:, m, :], ghn_ps[:, m, :],
                            wt["bghn" + tag][:, m:m + 1], rzs[:, m, :],
                            op0=ALU.add, op1=ALU.mult)
                        nc.vector.scalar_tensor_tensor(
                            t2[:, m, :], gin_ps[:, m, :],
                            wt["bgin" + tag][:, m:m + 1], t1[:, m, :],
                            op0=ALU.add, op1=ALU.add)
                    nc.scalar.activation(nn[:], t2[:], AF.Tanh)
                    # h' = n + z*(h-n), merged over m-tiles
                    nc.vector.tensor_sub(t1[:], hT[:], nn[:])
                    nc.vector.tensor_mul(t2[:], rzs[:, m_tiles:2 * m_tiles, :], t1[:])
                    nc.vector.tensor_add(hp[:], nn[:], t2[:])
                    return hp

                # -------- env pathway -----------------------------------
                pe1 = psA.tile([128, 2, chunk], F32, name="pe1", tag="psA")
                gemm(pe1, "wenv1", [xt[:, k, :] for k in range(12)], 2, chunk)
                eh1 = ev_pool.tile([128, 2, chunk], BF16, name="eh1")
                for m in range(2):
                    nc.scalar.activation(eh1[:, m, :], pe1[:, m, :], AF.Relu,
                                         bias=wt["benv1"][:, m:m + 1])

                pe2 = psA.tile([128, 2, chunk], F32, name="pe2", tag="psA")
                gemm(pe2, "wenv2", [eh1[:, k, :] for k in range(2)], 2, chunk)
                eh2 = ev_pool.tile([128, 2, chunk], BF16, name="eh2", bufs=2)
                for m in range(2):
                    nc.scalar.activation(eh2[:, m, :], pe2[:, m, :], AF.Identity,
                                         bias=wt["benv2"][:, m:m + 1])

                hTe = load_hT(0, "hTe")

                movs_rz = [eh2[:, 0, :], eh2[:, 1, :], hTe[:, 0, :], hTe[:, 1, :]]
                rz_ps = [psB.tile([128, chunk], F32, name="prz%d" % j, tag="psB")
                         for j in range(4)]
                for j in range(4):
                    for ki, mov in enumerate(movs_rz):
                        mm(rz_ps[j][:], wt["wrze"][:, ki, 128 * j:128 * (j + 1)],
                           mov, ki == 0, ki == 3)
                gin = psA.tile([128, 2, chunk], F32, name="gin", tag="psA")
                gemm(gin, "wgine", [eh2[:, k, :] for k in range(2)], 2, chunk)
                ghn = psA.tile([128, 2, chunk], F32, name="ghn", tag="psA")
                gemm(ghn, "wghne", [hTe[:, k, :] for k in range(2)], 2, chunk)

                henv = gru_tail("e", rz_ps, gin, ghn, hTe)
                pe_transpose_out(henv, h_out, b0, 0)

                # wo_q = henv @ W_wo.T + b_wo
                pwo = psA.tile([6, chunk], F32, name="pwo", tag="psA")
                for ki in range(2):
                    mm(pwo[:], wt["wwo"][:, ki, :], henv[:, ki, :], ki == 0, ki == 1)
                qwo = ev_pool.tile([6, chunk], BF16, name="qwo")
                nc.scalar.activation(qwo[:], pwo[:], AF.Identity, bias=wt["bwo"][:])

                # -------- enemies ---------------------------------------
                ams = []
                atts = []
                for t in range(N_EN):
                    tag = "a" if t < 8 else "b"
                    half = 64 * (t % 2)
                    fT = ft[half:half + 64, t // 2, :]

                    hTt = load_hT(H * (1 + t), "hTt")

                    ep1 = psA.tile([128, 2, chunk], F32, name="ep1", tag="psA")
                    for m in range(2):
                        mm(ep1[:, m, :],
                           wt["we1" + tag][half:half + 64, 128 * m:128 * (m + 1)],
                           fT, True, True)
                    e1r = ev_pool.tile([128, 2, chunk], BF16, name="e1r")
                    for m in range(2):
                        nc.scalar.activation(e1r[:, m, :], ep1[:, m, :], AF.Relu,
                                             bias=wt["be1" + tag][:, m:m + 1])

                    movs = [e1r[:, 0, :], e1r[:, 1, :], hTt[:, 0, :], hTt[:, 1, :]]
                    rz_ps = [psB.tile([128, chunk], F32, name="erz%d" % j, tag="psB")
                             for j in range(4)]
                    for j in range(4):
                        for ki, mov in enumerate(movs):
                            mm(rz_ps[j][:],
                               wt["wrz" + tag][:, ki, 128 * j:128 * (j + 1)],
                               mov, ki == 0, ki == 3)
                    gin = psA.tile([128, 2, chunk], F32, name="egin", tag="psA")
                    gemm(gin, "wgin" + tag, [e1r[:, k, :] for k in range(2)], 2, chunk)
                    ghn = psA.tile([128, 2, chunk], F32, name="eghn", tag="psA")
                    gemm(ghn, "wghn" + tag, [hTt[:, k, :] for k in range(2)], 2, chunk)

                    h3 = gru_tail(tag, rz_ps, gin, ghn, hTt)
                    pe_transpose_out(h3, h_out, b0, H * (1 + t))

                    am = am_pool.tile([128, 2, chunk], BF16, name="am")
                    nc.vector.tensor_mul(am[:], eh2[:], h3[:])
                    ams.append(am)

                    # attack reduction once each type's 8 enemies are done
                    if t % 8 == 7:
                        ty = t // 8
                        patt = psA.tile([8, chunk], F32, name="patt", tag="psA")
                        for j in range(8):
                            for k in range(2):
                                mm(patt[:], wt["sel"][:, 8 * j:8 * j + 8],
                                   ams[8 * ty + j][:, k, :],
                                   j == 0 and k == 0, j == 7 and k == 1)
                        att = ev_pool.tile([8, chunk], BF16, name="att")
                        nc.vector.tensor_copy(att[:], patt[:])
                        atts.append(att)

                # -------- q assembly (transpose + store) ----------------
                for s in range(ns):
                    cs = slice(128 * s, 128 * (s + 1))
                    pq1 = psT.tile([128, 6], BF16, name="pq1", tag="psT")
                    nc.tensor.transpose(pq1[:], qwo[:, cs], idt16[0:6, 0:6])
                    pq2 = psT.tile([128, 8], BF16, name="pq2", tag="psT")
                    nc.tensor.transpose(pq2[:], atts[0][:, cs], idt16[0:8, 0:8])
                    pq3 = psT.tile([128, 8], BF16, name="pq3", tag="psT")
                    nc.tensor.transpose(pq3[:], atts[1][:, cs], idt16[0:8, 0:8])
                    qsb = out_pool.tile([128, 22], F32, name="qsb")
                    nc.vector.tensor_copy(qsb[:, 0:6], pq1[:])
                    nc.vector.tensor_copy(qsb[:, 6:14], pq2[:])
                    nc.vector.tensor_copy(qsb[:, 14:22], pq3[:])
                    nc.sync.dma_start(out=q_out[b0 + 128 * s:b0 + 128 * (s + 1), :],
                                      in_=qsb[:])

    nc.compile()
    return nc


# --------------------------------------------------------------------------
# public entry point
# --------------------------------------------------------------------------
def _get_nc(bc, chunk):
    key = (bc, chunk)
    if key not in _CACHE:
        _CACHE[key] = build(bc, chunk)
    return _CACHE[key]


def run(inputs, bc=1024, chunk=512, trace=False):
    inputs = {k: np.asarray(v) for k, v in inputs.items()}
    nc = _get_nc(bc, chunk)
    w = prep_weights(inputs)
    x = np.ascontiguousarray(inputs["inputs"], np.float32)
    h = np.ascontiguousarray(inputs["hidden_state"], np.float32)
    n_cores = N_CORES
    assert x.shape[0] == bc * n_cores
    in_maps = []
    for i in range(n_cores):
        m = {"x": np.ascontiguousarray(x[i * bc:(i + 1) * bc]),
             "h": np.ascontiguousarray(h[i * bc:(i + 1) * bc])}
        m.update(w)
        in_maps.append(m)
    res = run_bass_kernel_spmd(nc, in_maps, list(range(n_cores)), trace=trace)
    q = np.concatenate([r["q"] for r in res.results], 0)
    hid = np.concatenate([r["hidden"] for r in res.results], 0)
    return (q, hid), res


def kernel(**inputs):
    (q, hid), _ = run(inputs, bc=B // N_CORES, chunk=512)
    return q, hid


# revision 21
# speedup vs baseline: 1.1899x; 1.1899x over previous
"""Trainium2 Bass kernel for nn_ArnDiffRnnAgent (dense_mlp, 8-core data parallel).

Strategy (v5)
-------------
Pure data parallel: batch 8192 split into 8 shards of 1024 rows, one per
NeuronCore; small weights replicated.  Activations live in SBUF
feature-major ([feature, batch]) so every GEMM maps directly onto the
TensorEngine.  Compute dtype bf16 with fp32 PSUM accumulation.

ALL layout conversion runs on the TensorEngine (PE transposes):
 - inputs:  f32 natural loads -> f32r PE-transpose (1.5 cyc/row) -> PSUM ->
   bf16 SBUF eviction on ScalarE/VectorE
 - outputs: bf16 PE-transpose (1 cyc/row) -> PSUM -> f32 SBUF eviction ->
   plain contiguous f32 DMA stores
The DMA engines only ever do clean contiguous loads/stores (the DMA-XBAR
transpose path measured ~150 GB/s effective, starved the PE, and corrupts
(col%16>=8, even-row) cells for these shapes).

Host-side weight prep (parameter layout prep, amortized in any real use):
 - weights pre-transposed/tiled to [128, k_tiles, M] bf16
 - enemy e2-layer folded into GRU input weights:
      gi = e2 @ wih.T + bih,  e2 = relu1 @ W_e2.T + b_e2
   => gi = relu1 @ (wih @ W_e2).T + (wih @ b_e2 + bih)
 - r/z gates of each GRU computed as one K=512 GEMM over [x_side | h_side]
"""
import sys
sys.path.insert(0, "/opt/trn_rl_repo")

import numpy as np
import ml_dtypes

import concourse.bass as bass  # noqa: F401
import concourse.tile as tile
from concourse import bacc, mybir
from concourse.bass_utils import run_bass_kernel_spmd

BF16 = mybir.dt.bfloat16
F32 = mybir.dt.float32
F32R = mybir.dt.float32r
AF = mybir.ActivationFunctionType
ALU = mybir.AluOpType
bf16 = ml_dtypes.bfloat16

N_CORES = 8
B = 8192
IN = 1504
H = 256
E = 64
MOVE = 16
N_EN = 16
HID = (1 + N_EN) * H  # 4352

_CACHE = {}


# --------------------------------------------------------------------------
# host-side weight prep
# --------------------------------------------------------------------------
def _ktile(wt):
    """[K, M] (f32) -> [128, K//128, M] bf16 host array (K padded to 128s)."""
    K = wt.shape[0]
    Kp = ((K + 127) // 128) * 128
    if Kp != K:
        wt = np.concatenate([wt, np.zeros((Kp - K, wt.shape[1]), np.float32)], 0)
    return np.ascontiguousarray(
        wt.reshape(Kp // 128, 128, wt.shape[1]).transpose(1, 0, 2)
    ).astype(bf16)


def _btile(b):
    """[M] f32 -> [128, M//128] f32."""
    return np.ascontiguousarray(b.reshape(-1, 128).T).astype(np.float32)


def prep_weights(i):
    w = {}
    f32 = np.float32
    # env layer1: k-tile 11 matches the overlapped transpose source
    # (x cols 1376:1504): rows 0:32 of that tile are zero (cols 1376:1408
    # are already covered by k-tile 10).
    w1 = i["W_env1"].astype(f32).T                           # [1504, 256]
    w1p = np.zeros((1536, 256), f32)
    w1p[0:1408] = w1[0:1408]
    w1p[1440:1536] = w1[1408:1504]
    w["wenv1"] = np.ascontiguousarray(
        w1p.reshape(12, 128, 256).transpose(1, 0, 2)).astype(bf16)
    w["benv1"] = _btile(i["b_env1"].astype(f32))
    w["wenv2"] = _ktile(i["W_env2"].astype(f32).T)           # [128,2,256]
    w["benv2"] = _btile(i["b_env2"].astype(f32))

    def gru(tag, wih, whh, bih, bhh):
        w["wrz" + tag] = _ktile(
            np.concatenate([wih[:512].T, whh[:512].T], 0))   # [128,4,512]
        w["brz" + tag] = _btile(bih[:512] + bhh[:512])
        w["wgin" + tag] = _ktile(wih[512:].T)                # [128,2,256]
        w["bgin" + tag] = _btile(bih[512:])
        w["wghn" + tag] = _ktile(whh[512:].T)
        w["bghn" + tag] = _btile(bhh[512:])

    gru("e", i["wih_env"].astype(f32), i["whh_env"].astype(f32),
        i["bih_env"].astype(f32), i["bhh_env"].astype(f32))

    for tag in ("a", "b"):
        wih = i["wih_e" + tag].astype(f32)
        whh = i["whh_e" + tag].astype(f32)
        bih = i["bih_e" + tag].astype(f32)
        bhh = i["bhh_e" + tag].astype(f32)
        We2 = i["W_e2" + tag].astype(f32)
        be2 = i["b_e2" + tag].astype(f32)
        Wgi = wih @ We2                                       # [768,256]
        bgi = wih @ be2 + bih
        gru(tag, Wgi, whh, bgi, bhh)
        # e1 weights duplicated in both partition halves (odd enemies sit at
        # base_partition 64 and matmul requires equal operand bases)
        e1t = i["W_e1" + tag].astype(f32).T                   # [64,256]
        w["we1" + tag] = np.ascontiguousarray(
            np.concatenate([e1t, e1t], 0)).astype(bf16)       # [128,256]
        w["be1" + tag] = _btile(i["b_e1" + tag].astype(f32))

    w["wwo"] = _ktile(i["W_wo"].astype(f32).T)                # [128,2,6]
    w["bwo"] = np.ascontiguousarray(i["b_wo"].astype(f32).reshape(6, 1))
    sel = np.zeros((128, 64), f32)
    for j in range(8):
        sel[:, j * 8 + j] = 1.0
    w["sel"] = sel.astype(bf16)
    w["ident"] = np.eye(128, dtype=f32)
    w["ident16"] = np.eye(128, dtype=f32).astype(bf16)
    return w


WEIGHT_SPECS = {
    "wenv1": ([128, 12, 256], BF16), "benv1": ([128, 2], F32),
    "wenv2": ([128, 2, 256], BF16), "benv2": ([128, 2], F32),
    "wwo": ([128, 2, 6], BF16), "bwo": ([6, 1], F32),
    "sel": ([128, 64], BF16), "ident": ([128, 128], F32R),
    "ident16": ([128, 128], BF16),
}
for _t in ("e", "a", "b"):
    WEIGHT_SPECS["wrz" + _t] = ([128, 4, 512], BF16)
    WEIGHT_SPECS["brz" + _t] = ([128, 4], F32)
    WEIGHT_SPECS["wgin" + _t] = ([128, 2, 256], BF16)
    WEIGHT_SPECS["bgin" + _t] = ([128, 2], F32)
    WEIGHT_SPECS["wghn" + _t] = ([128, 2, 256], BF16)
    WEIGHT_SPECS["bghn" + _t] = ([128, 2], F32)
for _t in ("a", "b"):
    WEIGHT_SPECS["we1" + _t] = ([128, 256], BF16)
    WEIGHT_SPECS["be1" + _t] = ([128, 2], F32)


# --------------------------------------------------------------------------
# device kernel builder
# --------------------------------------------------------------------------
def build(bc=1024, chunk=512):
    """bc = batch rows per core, chunk = moving free-dim per GEMM."""
    nc = bacc.Bacc("TRN2", target_bir_lowering=False, debug=False,
                   num_devices=N_CORES)

    ns = chunk // 128  # b-subtiles per chunk

    x = nc.dram_tensor("x", [bc, IN], F32, kind="ExternalInput")
    h = nc.dram_tensor("h", [bc, HID], F32, kind="ExternalInput")
    W = {k: nc.dram_tensor(k, s, d, kind="ExternalInput")
         for k, (s, d) in WEIGHT_SPECS.items()}

    q_out = nc.dram_tensor("q", [bc, 22], F32, kind="ExternalOutput")
    h_out = nc.dram_tensor("hidden", [bc, HID], F32, kind="ExternalOutput")

    nch = bc // chunk

    with tile.TileContext(nc) as tc:
        with (
            tc.tile_pool(name="wts", bufs=1) as wpool,
            tc.tile_pool(name="xn", bufs=1) as xn_pool,
            tc.tile_pool(name="hn", bufs=5) as hn_pool,
            tc.tile_pool(name="xt", bufs=2) as xt_pool,
            tc.tile_pool(name="ft", bufs=2) as ft_pool,
            tc.tile_pool(name="ht", bufs=5) as ht_pool,
            tc.tile_pool(name="ev", bufs=3) as ev_pool,
            tc.tile_pool(name="am", bufs=10) as am_pool,
            tc.tile_pool(name="outp", bufs=3) as out_pool,
            tc.tile_pool(name="psT", bufs=2, space="PSUM") as psT,
            tc.tile_pool(name="psA", bufs=2, space="PSUM") as psA,
            tc.tile_pool(name="psB", bufs=2, space="PSUM") as psB,
        ):
            # ---- weights into SBUF -------------------------------------
            wt = {}
            for k, (s, d) in WEIGHT_SPECS.items():
                wt[k] = wpool.tile(s, d, name="w_" + k)
                nc.sync.dma_start(wt[k][:], W[k][:])
            idt = wt["ident"]
            idt16 = wt["ident16"]

            def mm(psum_ap, lhs_ap, mov_ap, start, stop):
                nc.tensor.matmul(psum_ap, lhs_ap, mov_ap, start=start, stop=stop)

            def gemm(ps_tile, wkey, movs, m_tiles, n):
                wtile = wt[wkey]
                nk = len(movs)
                for m in range(m_tiles):
                    for ki, mov in enumerate(movs):
                        mm(ps_tile[:, m, :],
                           wtile[:, ki, 128 * m:128 * (m + 1)],
                           mov, ki == 0, ki == nk - 1)

            evict_ctr = [0]

            def evict(dst_ap, src_ap):
                evict_ctr[0] += 1
                if evict_ctr[0] % 2 == 0:
                    nc.scalar.activation(dst_ap, src_ap, AF.Identity)
                else:
                    nc.vector.tensor_copy(dst_ap, src_ap)

            # input transpose: ns x [128,128] f32r tiles -> bf16 [128, chunk]
            def pe_transpose_in(col_fn, dst_ap):
                pt = psT.tile([128, ns, 128], BF16, name="pt", tag="psT")
                for s in range(ns):
                    nc.tensor.transpose(pt[:, s, :], col_fn(s), idt16[:])
                evict(dst_ap, pt[:])

            # output transpose: src [128, 2, chunk] bf16 feature-major ->
            # f32 stores into out_dram[b0:b0+chunk, c0:c0+256]
            def pe_transpose_out(src, out_dram, b0, c0):
                for w0 in range(0, ns, 2):
                    sp = min(2, ns - w0)
                    po = psT.tile([128, 2, 2, 128], BF16, name="po", tag="psT")
                    for si in range(sp):
                        for k in range(2):
                            nc.tensor.transpose(
                                po[:, si, k, :],
                                src[:, k, 128 * (w0 + si):128 * (w0 + si + 1)],
                                idt16[:])
                    ho = out_pool.tile([128, 2, 256], F32, name="ho")
                    evict(ho[:, 0:sp, :], po[:, 0:sp, :, :])
                    r0 = b0 + 128 * w0
                    nc.sync.dma_start(
                        out=out_dram[r0:r0 + 128 * sp,
                                     c0:c0 + 256].rearrange(
                            "(s p) c -> p s c", p=128),
                        in_=ho[:, 0:sp, :])

            # ---- main loop ---------------------------------------------
            for c in range(nch):
                b0 = c * chunk

                # natural-layout f32 loads
                xn = xn_pool.tile([128, ns, IN], BF16, name="xn")
                for s in range(ns):
                    nc.gpsimd.dma_start(
                        out=xn[:, s, :],
                        in_=x[b0 + 128 * s:b0 + 128 * (s + 1), :])

                # transposed env input: [128, 12, chunk]
                xt = xt_pool.tile([128, 12, chunk], BF16, name="xt")
                for k in range(12):
                    c0 = 128 * k if k < 11 else IN - 128
                    pe_transpose_in(lambda s, c0=c0: xn[:, s, c0:c0 + 128],
                                    xt[:, k, :])

                # enemy features: 8 tiles covering 2 enemies each
                ft = ft_pool.tile([128, 8, chunk], BF16, name="ft")
                for j in range(8):
                    c0 = MOVE + 128 * j
                    pe_transpose_in(lambda s, c0=c0: xn[:, s, c0:c0 + 128],
                                    ft[:, j, :])

                def load_hT(col0, name):
                    hn = hn_pool.tile([128, ns, H], BF16, name="hn")
                    for s in range(ns):
                        nc.gpsimd.dma_start(
                            out=hn[:, s, :],
                            in_=h[b0 + 128 * s:b0 + 128 * (s + 1),
                                  col0:col0 + H])
                    hT = ht_pool.tile([128, 2, chunk], BF16, name=name)
                    for k in range(2):
                        pe_transpose_in(
                            lambda s, k=k: hn[:, s, 128 * k:128 * (k + 1)],
                            hT[:, k, :])
                    return hT

                # -------- shared GRU tail -------------------------------
                def gru_tail(tag, rz_ps, gin_ps, ghn_ps, hT, m_tiles=2):
                    rzs = ev_pool.tile([128, 2 * m_tiles, chunk], BF16, name="rzs")
                    for j in range(2 * m_tiles):
                        nc.scalar.activation(
                            rzs[:, j, :], rz_ps[j][:], AF.Sigmoid,
                            bias=wt["brz" + tag][:, j:j + 1])
                    t1 = ev_pool.tile([128, m_tiles, chunk], BF16, name="t1")
                    t2 = ev_pool.tile([128, m_tiles, chunk], BF16, name="t2")
                    nn = ev_pool.tile([128, m_tiles, chunk], BF16, name="nn")
                    hp = ev_pool.tile([128, m_tiles, chunk], BF16, name="hp")
                    for m in range(m_tiles):
                        nc.vector.scalar_tensor_tensor(
                            t1[:, m, :], ghn_ps[:, m, :],
                            wt["bghn" + tag][:, m:m + 1], rzs[:, m, :],
                            op0=ALU.add, op1=ALU.mult)
                        nc.vector.scalar_tensor_tensor(
                            t2[:, m, :], gin_ps[:, m, :],
                            wt["bgin" + tag][:, m:m + 1], t1[:, m, :],
                            op0=ALU.add, op1=ALU.add)
                    nc.scalar.activation(nn[:], t2[:], AF.Tanh)
                    # h' = n + z*(h-n), merged over m-tiles
                    nc.vector.tensor_sub(t1[:], hT[:], nn[:])
                    nc.vector.tensor_mul(t2[:], rzs[:, m_tiles:2 * m_tiles, :], t1[:])
                    nc.vector.tensor_add(hp[:], nn[:], t2[:])
                    return hp

                # -------- env pathway -----------------------------------
                pe1 = psA.tile([128, 2, chunk], F32, name="pe1", tag="psA")
                gemm(pe1, "wenv1", [xt[:, k, :] for k in range(12)], 2, chunk)
                eh1 = ev_pool.tile([128, 2, chunk], BF16, name="eh1")
                for m in range(2):
                    nc.scalar.activation(eh1[:, m, :], pe1[:, m, :], AF.Relu,
                                         bias=wt["benv1"][:, m:m + 1])

                pe2 = psA.tile([128, 2, chunk], F32, name="pe2", tag="psA")
                gemm(pe2, "wenv2", [eh1[:, k, :] for k in range(2)], 2, chunk)
                eh2 = ev_pool.tile([128, 2, chunk], BF16, name="eh2", bufs=2)
                for m in range(2):
                    nc.scalar.activation(eh2[:, m, :], pe2[:, m, :], AF.Identity,
                                         bias=wt["benv2"][:, m:m + 1])

                hTe = load_hT(0, "hTe")

                movs_rz = [eh2[:, 0, :], eh2[:, 1, :], hTe[:, 0, :], hTe[:, 1, :]]
                rz_ps = [psB.tile([128, chunk], F32, name="prz%d" % j, tag="psB")
                         for j in range(4)]
                for j in range(4):
                    for ki, mov in enumerate(movs_rz):
                        mm(rz_ps[j][:], wt["wrze"][:, ki, 128 * j:128 * (j + 1)],
                           mov, ki == 0, ki == 3)
                gin = psA.tile([128, 2, chunk], F32, name="gin", tag="psA")
                gemm(gin, "wgine", [eh2[:, k, :] for k in range(2)], 2, chunk)
                ghn = psA.tile([128, 2, chunk], F32, name="ghn", tag="psA")
                gemm(ghn, "wghne", [hTe[:, k, :] for k in range(2)], 2, chunk)

                henv = gru_tail("e", rz_ps, gin, ghn, hTe)
                pe_transpose_out(henv, h_out, b0, 0)

                # wo_q = henv @ W_wo.T + b_wo
                pwo = psA.tile([6, chunk], F32, name="pwo", tag="psA")
                for ki in range(2):
                    mm(pwo[:], wt["wwo"][:, ki, :], henv[:, ki, :], ki == 0, ki == 1)
                qwo = ev_pool.tile([6, chunk], BF16, name="qwo")
                nc.scalar.activation(qwo[:], pwo[:], AF.Identity, bias=wt["bwo"][:])

                # -------- enemies ---------------------------------------
                ams = []
                atts = []
                for t in range(N_EN):
                    tag = "a" if t < 8 else "b"
                    half = 64 * (t % 2)
                    fT = ft[half:half + 64, t // 2, :]

                    hTt = load_hT(H * (1 + t), "hTt")

                    ep1 = psA.tile([128, 2, chunk], F32, name="ep1", tag="psA")
                    for m in range(2):
                        mm(ep1[:, m, :],
                           wt["we1" + tag][half:half + 64, 128 * m:128 * (m + 1)],
                           fT, True, True)
                    e1r = ev_pool.tile([128, 2, chunk], BF16, name="e1r")
                    for m in range(2):
                        nc.scalar.activation(e1r[:, m, :], ep1[:, m, :], AF.Relu,
                                             bias=wt["be1" + tag][:, m:m + 1])

                    movs = [e1r[:, 0, :], e1r[:, 1, :], hTt[:, 0, :], hTt[:, 1, :]]
                    rz_ps = [psB.tile([128, chunk], F32, name="erz%d" % j, tag="psB")
                             for j in range(4)]
                    for j in range(4):
                        for ki, mov in enumerate(movs):
                            mm(rz_ps[j][:],
                               wt["wrz" + tag][:, ki, 128 * j:128 * (j + 1)],
                               mov, ki == 0, ki == 3)
                    gin = psA.tile([128, 2, chunk], F32, name="egin", tag="psA")
                    gemm(gin, "wgin" + tag, [e1r[:, k, :] for k in range(2)], 2, chunk)
                    ghn = psA.tile([128, 2, chunk], F32, name="eghn", tag="psA")
                    gemm(ghn, "wghn" + tag, [hTt[:, k, :] for k in range(2)], 2, chunk)

                    h3 = gru_tail(tag, rz_ps, gin, ghn, hTt)
                    pe_transpose_out(h3, h_out, b0, H * (1 + t))

                    am = am_pool.tile([128, 2, chunk], BF16, name="am")
                    nc.vector.tensor_mul(am[:], eh2[:], h3[:])
                    ams.append(am)

                    # attack reduction once each type's 8 enemies are done
                    if t % 8 == 7:
                        ty = t // 8
                        patt = psA.tile([8, chunk], F32, name="patt", tag="psA")
                        for j in range(8):
                            for k in range(2):
                                mm(patt[:], wt["sel"][:, 8 * j:8 * j + 8],
                                   ams[8 * ty + j][:, k, :],
                                   j == 0 and k == 0, j == 7 and k == 1)
                        att = ev_pool.tile([8, chunk], BF16, name="att")
                        nc.vector.tensor_copy(att[:], patt[:])
                        atts.append(att)

                # -------- q assembly (transpose + store) ----------------
                for s in range(ns):
                    cs = slice(128 * s, 128 * (s + 1))
                    pq1 = psT.tile([128, 6], BF16, name="pq1", tag="psT")
                    nc.tensor.transpose(pq1[:], qwo[:, cs], idt16[0:6, 0:6])
                    pq2 = psT.tile([128, 8], BF16, name="pq2", tag="psT")
                    nc.tensor.transpose(pq2[:], atts[0][:, cs], idt16[0:8, 0:8])
                    pq3 = psT.tile([128, 8], BF16, name="pq3", tag="psT")
                    nc.tensor.transpose(pq3[:], atts[1][:, cs], idt16[0:8, 0:8])
                    qsb = out_pool.tile([128, 22], F32, name="qsb")
                    nc.vector.tensor_copy(qsb[:, 0:6], pq1[:])
                    nc.vector.tensor_copy(qsb[:, 6:14], pq2[:])
                    nc.vector.tensor_copy(qsb[:, 14:22], pq3[:])
                    nc.sync.dma_start(out=q_out[b0 + 128 * s:b0 + 128 * (s + 1), :],
                                      in_=qsb[:])

    nc.compile()
    return nc


# --------------------------------------------------------------------------
# public entry point
# --------------------------------------------------------------------------
def _get_nc(bc, chunk):
    key = (bc, chunk)
    if key not in _CACHE:
        _CACHE[key] = build(bc, chunk)
    return _CACHE[key]


def run(inputs, bc=1024, chunk=512, trace=False):
    inputs = {k: np.asarray(v) for k, v in inputs.items()}
    nc = _get_nc(bc, chunk)
    w = prep_weights(inputs)
    x = np.ascontiguousarray(inputs["inputs"], np.float32)
    h = np.ascontiguousarray(inputs["hidden_state"], np.float32)
    n_cores = N_CORES
    assert x.shape[0] == bc * n_cores
    in_maps = []
    for i in range(n_cores):
        m = {"x": np.ascontiguousarray(x[i * bc:(i + 1) * bc]),
             "h": np.ascontiguousarray(h[i * bc:(i + 1) * bc])}
        m.update(w)
        in_maps.append(m)
    res = run_bass_kernel_spmd(nc, in_maps, list(range(n_cores)), trace=trace)
    q = np.concatenate([r["q"] for r in res.results], 0)
    hid = np.concatenate([r["hidden"] for r in res.results], 0)
    return (q, hid), res


def kernel(**inputs):
    (q, hid), _ = run(inputs, bc=B // N_CORES, chunk=512)
    return q, hid
